# revision 25
# baseline (speedup 1.0000x reference)
"""ChamferLoss2D on 8 Trainium2 NeuronCores.

Data parallel: batch n -> core n. Each core computes, for its (4096,2)
point sets x,y, the full 4096x4096 squared-distance matrix via a single
K-row matmul using the norm expansion

    D[i,j] = ||x_i||^2 + ||y_j||^2 - 2 x_i . y_j

then min-reduces it along both axes, sqrts the 2*4096 minima (min and
sqrt commute on nonneg values), means, and averages.

Numerics: the norm expansion cancels catastrophically (D_min ~ 1e-4 vs
terms ~ 1-10), so matmul operands must carry ~fp32 precision. The PE
runs fp32 at 1/4 rate, so instead each fp32 operand row is split
exactly into hi+lo float16 pieces (11-bit mantissa each => products
carry ~22-bit precision) and the matmul runs with K=10 fp16 rows at
full PE speed:

    (xh+xl)(sh+sl) ~ xh.sh + xh.sl + xl.sh        (3 rows per coord)
    norms: (xnh + xnl) * 1, 1 * (ynh + ynl)       (4 rows)

Coordinates are pre-scaled by 16 (exact power of two, folded out of the
final result) so that fp16 subnormal flushing of lo pieces is
negligible relative to D_min.

Engine split, 8 groups of 4 128-row chunks:
  PE    matmuls [10,128]x[10,512] -> PSUM (4-bank tiles)
  ACT   converts [128,2048] PSUM f32 -> SBUF bf16 rows
  DVE   column-min accumulation (tensor_tensor min, bf16 2x mode) and
        row-min in-place fold trees over [128,4,4096] groups
Cross-partition minima of the two accumulated colacc halves: half A is
DMA-xbar-transposed in 128x128 blocks overlapping the second half of
the main loop (its reduce is emitted a few groups later so the in-order
DVE stream never waits on the transposes); half B is PE-transposed
(identity matmul) in the tail, where the PE is idle. Final partition
sum via matmul against ones.
"""

import os
from contextlib import ExitStack

import numpy as np

import concourse.bass as bass
import concourse.tile as tile
from concourse import bacc, mybir
from concourse.bass_utils import run_bass_kernel_spmd

F32 = mybir.dt.float32
F16 = mybir.dt.float16
BF16 = mybir.dt.bfloat16

P = 4096          # points per set
NCORES = 8
IC = 128          # rows per i-chunk (matmul M)
NI = P // IC      # 32 i-chunks
GRP = 4           # chunks per rowmin fold group
JW = 2048         # columns per psum tile (4 PSUM banks)
NJ = P // JW      # psum tiles per chunk
MMN = 512         # matmul moving free dim (1 bank fp32)
NB = P // IC      # 32 transpose blocks per colacc

SC = 16.0         # coordinate prescale (power of two)
BIG = 1.0e30
MIN_OP = mybir.AluOpType.min
ADD_OP = mybir.AluOpType.add
AX = mybir.AxisListType.X
SQRT = mybir.ActivationFunctionType.Sqrt


def _build_side(nc, wide, raw, coord_scale, mm_mode, placement, K):
    """Load one (P,2) point set and compute its operand row pieces into
    one wide [32, K*128] buffer (piece for operand row r at slice r).

    Wide layout: partition a holds points a*128..a*128+127. Returns the
    filled piece buffer. `placement` maps piece name -> list of rows.
    """
    ODT = F32 if mm_mode == "f32" else F16
    pb = wide.tile([32, K * IC], ODT, name=f"pb_{raw.name}")

    def sl(r):
        return pb[:, r * IC : (r + 1) * IC]

    def put(name, ap_writer):
        rows_ = placement[name]
        ap_writer(sl(rows_[0]))
        for r in rows_[1:]:
            nc.scalar.copy(sl(r), sl(rows_[0]))

    # constant-one rows (adjacent by construction) via one memset
    ones_rows = placement["one"]
    nc.gpsimd.memset(
        pb[:, ones_rows[0] * IC : (ones_rows[-1] + 1) * IC], 1.0
    )

    xw = wide.tile([32, 2 * IC], F32, name=f"xw_{raw.name}")
    nc.sync.dma_start(xw[:], raw.rearrange("(a b) c -> a (b c)", a=32))
    xp = wide.tile([32, 2 * IC], F32, name=f"xp_{raw.name}")
    nc.scalar.mul(xp[:], xw[:], coord_scale)

    xpv = xp[:].rearrange("p (q c) -> p c q", c=2)   # [32, 2, 128]

    sq0 = wide.tile([32, IC], F32, name=f"sq0_{raw.name}")
    nc.vector.tensor_mul(sq0[:], xpv[:, 0, :], xpv[:, 0, :])
    sq1 = wide.tile([32, IC], F32, name=f"sq1_{raw.name}")
    nc.vector.tensor_mul(sq1[:], xpv[:, 1, :], xpv[:, 1, :])
    xn = wide.tile([32, IC], F32, name=f"xn_{raw.name}")
    nc.vector.tensor_add(xn[:], sq0[:], sq1[:])
    norm_fix = (SC / coord_scale) ** 2
    if norm_fix != 1.0:
        nc.scalar.mul(xn[:], xn[:], norm_fix)

    if mm_mode == "f32":
        put("c0", lambda d: nc.vector.tensor_copy(d, xpv[:, 0, :]))
        put("c1", lambda d: nc.vector.tensor_copy(d, xpv[:, 1, :]))
        put("n", lambda d: nc.vector.tensor_copy(d, xn[:]))
        return pb

    for c in (0, 1):
        put(f"c{c}h", lambda d, c=c: nc.scalar.copy(d, xpv[:, c, :]))
        h = sl(placement[f"c{c}h"][0])
        dd = wide.tile([32, IC], F32, name=f"c{c}d_{raw.name}")
        nc.vector.tensor_sub(dd[:], xpv[:, c, :], h)
        put(f"c{c}l", lambda d, dd=dd: nc.scalar.copy(d, dd[:]))
    put("nh", lambda d: nc.scalar.copy(d, xn[:]))
    nh = sl(placement["nh"][0])
    nd = wide.tile([32, IC], F32, name=f"nd_{raw.name}")
    nc.vector.tensor_sub(nd[:], xn[:], nh)
    put("nl", lambda d: nc.scalar.copy(d, nd[:]))
    return pb


class _DmaRR:
    """Round-robin DMAs over the SP HWDGE ring and the GPSIMD SWDGE
    ring. Never issues DMA from the scalar engine: a dma_start in the
    ACT instruction stream blocks the conversions behind it."""

    def __init__(self, nc):
        self.engines = [nc.sync, nc.scalar]
        self.i = 0

    def dma(self, out, in_, **kw):
        e = self.engines[self.i % len(self.engines)]
        self.i += 1
        e.dma_start(out, in_, **kw)


def _gather_side(nc, rr, dram_pool, pb, flat4, K):
    """Bounce the piece buffer through DRAM, landing it as flat operand
    rows [K, P] at partition strip 0, then replicate to strips
    32/64/96 for row-packed matmuls. Processed in two point-halves so
    the first half's chain completes early."""
    stage = dram_pool.tile([32, K * IC], pb.dtype, name=f"stage_{pb.name}")
    rr.dma(stage[:], pb[:])
    rr.dma(
        flat4[0:K, :],
        stage[:].rearrange("a (r q) -> r a q", r=K),
    )
    for t in range(1, 4):
        rr.dma(flat4[32 * t : 32 * t + K, :], flat4[0:K, :])


def build(mm_mode="fp16x2"):
    nc = bacc.Bacc(
        "TRN2", target_bir_lowering=False, debug=False, num_devices=NCORES
    )
    x = nc.dram_tensor("x", [P, 2], F32, kind="ExternalInput").ap()
    y = nc.dram_tensor("y", [P, 2], F32, kind="ExternalInput").ap()
    ident = nc.dram_tensor("ident", [IC, IC], BF16, kind="ExternalInput").ap()
    out_d = nc.dram_tensor("out", [1, 1], F32, kind="ExternalOutput").ap()

    K = 4 if mm_mode == "f32" else 10
    ODT = F32 if mm_mode == "f32" else F16
    NG = NI // GRP            # 8 groups
    HALF_G = NG // 2          # group index where colacc half 0 completes
    CM0_G = NG - 2            # emit half-0 reduce during this group

    with ExitStack() as ctx:
        tc = ctx.enter_context(tile.TileContext(nc))
        konst = ctx.enter_context(tc.tile_pool(name="konst", bufs=1))
        wide = ctx.enter_context(tc.tile_pool(name="wide", bufs=1))
        dram = ctx.enter_context(tc.tile_pool(name="dram", bufs=1, space="DRAM"))
        psum = ctx.enter_context(tc.tile_pool(name="psum", bufs=2, space="PSUM"))
        rows = ctx.enter_context(tc.tile_pool(name="rows", bufs=3))
        accp = ctx.enter_context(tc.tile_pool(name="accp", bufs=1))
        smalls = ctx.enter_context(tc.tile_pool(name="smalls", bufs=1))

        rr = _DmaRR(nc)

        # preload the sqrt activation table while input DMAs run
        dum = smalls.tile([1, 1], F32, name="dum")
        nc.gpsimd.memset(dum[:], 1.0)
        dum2 = smalls.tile([1, 1], F32, name="dum2")
        nc.scalar.activation(dum2[:], dum[:], SQRT)

        idt = konst.tile([IC, IC], BF16, name="idt")
        rr.dma(idt[:], ident)

        # operands replicated at partition strips 0/32/64/96 so four
        # small-K matmuls run concurrently in the PE's 32-row groups
        lhsT4 = konst.tile([IC, P], ODT, name="lhsT4")
        rhs4 = konst.tile([IC, P], ODT, name="rhs4")
        lhsT = lhsT4[0:K, :]
        rhs = rhs4[0:K, :]
        colacc = [accp.tile([IC, P], BF16, name=f"colacc{h}") for h in (0, 1)]
        nc.gpsimd.memset(colacc[0][:], BIG)
        nc.gpsimd.memset(colacc[1][:], BIG)
        rmins = smalls.tile([IC, NI], F32, name="rmins")

        if mm_mode == "f32":
            # lhsT rows: [x0, x1, xn, 1]; rhs rows: [s0, s1, 1, yn]
            lplace = {"c0": [0], "c1": [1], "n": [2], "one": [3]}
            rplace = {"c0": [0], "c1": [1], "one": [2], "n": [3]}
        else:
            # lhsT rows: [x0h x0h x0l  x1h x1h x1l  xnh xnl  1 1]
            # rhs  rows: [s0h s0l s0h  s1h s1l s1h  1   1    ynh ynl]
            lplace = {"c0h": [0, 1], "c0l": [2], "c1h": [3, 4], "c1l": [5],
                      "nh": [6], "nl": [7], "one": [8, 9]}
            rplace = {"c0h": [0, 2], "c0l": [1], "c1h": [3, 5], "c1l": [4],
                      "one": [6, 7], "nh": [8], "nl": [9]}
        xpb = _build_side(nc, wide, x, SC, mm_mode, lplace, K)
        ypb = _build_side(nc, wide, y, -2.0 * SC, mm_mode, rplace, K)
        _gather_side(nc, rr, dram, xpb, lhsT4, K)
        _gather_side(nc, rr, dram, ypb, rhs4, K)

        ct0 = accp.tile([IC, P], BF16, name="ct0")
        cm0 = smalls.tile([IC, NB], F32, name="cm0")

        # group schedule: 4-chunk groups, except the last two groups are
        # 2 chunks each so the end-of-loop DVE drain is shorter
        gsizes = [GRP] * (NI // GRP - 1) + [GRP // 2, GRP // 2]
        assert sum(gsizes) == NI

        # ---- main loop ----
        ic = 0
        for g, gs in enumerate(gsizes):
            g0 = ic
            drow = rows.tile([IC, GRP, P], BF16, name="drow")
            for cg in range(gs):
                half = (2 * ic) // NI
                for jc in range(NJ):
                    pt = psum.tile([IC, JW], F32, name="pt")
                    # 4 concurrent matmuls in distinct 32-row PE strips
                    for h in range(JW // MMN):
                        j0 = jc * JW + h * MMN
                        nc.tensor.matmul(
                            pt[:, h * MMN : (h + 1) * MMN],
                            lhsT4[32 * h : 32 * h + K,
                                  ic * IC : (ic + 1) * IC],
                            rhs4[32 * h : 32 * h + K, j0 : j0 + MMN],
                            start=True,
                            stop=True,
                            tile_position=(32 * h, 0),
                        )
                    nc.scalar.copy(
                        drow[:, cg, jc * JW : (jc + 1) * JW], pt[:]
                    )
                # column-min accumulation for this chunk (bf16 2x)
                nc.vector.tensor_tensor(
                    colacc[half][:], colacc[half][:], drow[:, cg, :], op=MIN_OP
                )
                ic += 1
                # half 0 complete -> xbar-transpose it on the DMA engines,
                # overlapping the second half of the loop
                if ic == NI // 2:
                    for b in range(NB):
                        bs = slice(b * IC, (b + 1) * IC)
                        nc.sync.dma_start(ct0[:, bs], colacc[0][:, bs],
                                          transpose=True)
            # row min: in-place halving fold tree over the whole group
            w = P
            while w > 32:
                w //= 2
                nc.vector.tensor_tensor(
                    drow[:, :gs, :w], drow[:, :gs, :w],
                    drow[:, :gs, w : 2 * w], op=MIN_OP,
                )
            nc.vector.tensor_reduce(
                rmins[:, g0 : g0 + gs], drow[:, :gs, :32],
                axis=AX, op=MIN_OP,
            )

        # ---- tail: half 0 reduce (transposes done long ago) runs on
        # DVE while the PE transposes half 1; half-1 reduces pipelined
        # in 8-block batches behind the transposes ----
        ct0v = ct0[:].rearrange("p (b q) -> p b q", q=IC)
        w = IC
        while w > 16:
            w //= 2
            nc.vector.tensor_tensor(
                ct0v[:, :, :w], ct0v[:, :, :w], ct0v[:, :, w : 2 * w],
                op=MIN_OP,
            )
        nc.vector.tensor_reduce(
            cm0[:], ct0v[:, :, 0:16], axis=AX, op=MIN_OP,
        )
        ctb = psum.tile([IC, P], BF16, name="pt")
        cm1 = smalls.tile([IC, NB], F32, name="cm1")
        BB = 8
        for b0 in range(0, NB, BB):
            for b in range(b0, b0 + BB):
                bs = slice(b * IC, (b + 1) * IC)
                nc.tensor.transpose(ctb[:, bs], colacc[1][:, bs], idt[:])
            nc.vector.tensor_reduce(
                cm1[:, b0 : b0 + BB],
                ctb[:, b0 * IC : (b0 + BB) * IC].rearrange(
                    "p (b q) -> p b q", q=IC),
                axis=AX, op=MIN_OP,
            )
        cmin = smalls.tile([IC, NB], F32, name="cmin")
        nc.vector.tensor_tensor(cmin[:], cm0[:], cm1[:], op=MIN_OP)

        # ---- finalize: clamp, sqrt, row sums ----
        rclamp = smalls.tile([IC, NI], F32, name="rclamp")
        nc.vector.tensor_scalar_max(rclamp[:], rmins[:], 0.0)
        rsq = smalls.tile([IC, NI], F32, name="rsq")
        nc.scalar.activation(rsq[:], rclamp[:], SQRT)
        rsum = smalls.tile([IC, 1], F32, name="rsum")
        nc.vector.tensor_reduce(rsum[:], rsq[:], axis=AX, op=ADD_OP)

        cclamp = smalls.tile([IC, NB], F32, name="cclamp")
        nc.vector.tensor_scalar_max(cclamp[:], cmin[:], 0.0)
        csq = smalls.tile([IC, NB], F32, name="csq")
        nc.scalar.activation(csq[:], cclamp[:], SQRT)
        csum = smalls.tile([IC, 1], F32, name="csum")
        nc.vector.tensor_reduce(csum[:], csq[:], axis=AX, op=ADD_OP)

        # ---- combine: partition-sum via matmul with ones, then scale ----
        both = smalls.tile([IC, 1], F32, name="both")
        nc.vector.tensor_add(both[:], rsum[:], csum[:])
        ones = smalls.tile([IC, 1], F32, name="ones")
        nc.gpsimd.memset(ones[:], 1.0)
        fin_ps = psum.tile([IC, JW], F32, name="pt")
        nc.tensor.matmul(fin_ps[0:1, 0:1], both[:], ones[:], start=True,
                         stop=True)
        fin = smalls.tile([1, 1], F32, name="fin")
        nc.scalar.mul(fin[:], fin_ps[0:1, 0:1], 1.0 / (SC * 2.0 * P))
        nc.sync.dma_start(out_d, fin[:])

    nc.compile()
    return nc


_NC_CACHE = {}


def _get_nc(mm_mode):
    if mm_mode not in _NC_CACHE:
        _NC_CACHE[mm_mode] = build(mm_mode)
    return _NC_CACHE[mm_mode]


def _ident_np():
    import ml_dtypes
    return np.eye(IC, dtype=ml_dtypes.bfloat16)


def run(point_set_1, point_set_2, mm_mode=None, trace=False, tmpdir=None):
    """Run on 8 cores; returns ((8,) result, BassKernelResults)."""
    mm_mode = mm_mode or os.environ.get("CHAMFER_MM_MODE", "fp16x2")
    nc = _get_nc(mm_mode)
    x = np.ascontiguousarray(np.asarray(point_set_1), dtype=np.float32)
    y = np.ascontiguousarray(np.asarray(point_set_2), dtype=np.float32)
    assert x.shape == (NCORES, P, 2) and y.shape == (NCORES, P, 2)
    ident = _ident_np()
    in_maps = [{"x": x[c], "y": y[c], "ident": ident} for c in range(NCORES)]
    res = run_bass_kernel_spmd(
        nc, in_maps, list(range(NCORES)), trace=trace, tmpdir=tmpdir
    )
    out = np.array(
        [res.results[c]["out"][0, 0] for c in range(NCORES)], dtype=np.float32
    )
    return out, res


def kernel(point_set_1, point_set_2):
    out, _ = run(point_set_1, point_set_2)
    return out


# revision 26
# speedup vs baseline: 1.1755x; 1.1755x over previous
"""ChamferLoss2D on 8 Trainium2 NeuronCores.

Data parallel: batch n -> core n. Each core computes, for its (4096,2)
point sets x,y, the full 4096x4096 squared-distance matrix via a single
K-row matmul using the norm expansion

    D[i,j] = ||x_i||^2 + ||y_j||^2 - 2 x_i . y_j

then min-reduces it along both axes, sqrts the 2*4096 minima (min and
sqrt commute on nonneg values), means, and averages.

Numerics: the norm expansion cancels catastrophically (D_min ~ 1e-4 vs
terms ~ 1-10), so matmul operands must carry ~fp32 precision. The PE
runs fp32 at 1/4 rate, so instead each fp32 operand row is split
exactly into hi+lo float16 pieces (11-bit mantissa each => products
carry ~22-bit precision) and the matmul runs with K=10 fp16 rows at
full PE speed:

    (xh+xl)(sh+sl) ~ xh.sh + xh.sl + xl.sh        (3 rows per coord)
    norms: (xnh + xnl) * 1, 1 * (ynh + ynl)       (4 rows)

Coordinates are pre-scaled by 16 (exact power of two, folded out of the
final result) so that fp16 subnormal flushing of lo pieces is
negligible relative to D_min.

Engine split, 8 groups of 4 128-row chunks:
  PE    matmuls [10,128]x[10,512] -> PSUM (4-bank tiles)
  ACT   converts [128,2048] PSUM f32 -> SBUF bf16 rows
  DVE   column-min accumulation (tensor_tensor min, bf16 2x mode) and
        row-min in-place fold trees over [128,4,4096] groups
Cross-partition minima of the two accumulated colacc halves: half A is
DMA-xbar-transposed in 128x128 blocks overlapping the second half of
the main loop (its reduce is emitted a few groups later so the in-order
DVE stream never waits on the transposes); half B is PE-transposed
(identity matmul) in the tail, where the PE is idle. Final partition
sum via matmul against ones.
"""

import os
from contextlib import ExitStack

import numpy as np

import concourse.bass as bass
import concourse.tile as tile
from concourse import bacc, mybir
from concourse.bass_utils import run_bass_kernel_spmd

F32 = mybir.dt.float32
F16 = mybir.dt.float16
BF16 = mybir.dt.bfloat16

P = 4096          # points per set
NCORES = 8
IC = 128          # rows per i-chunk (matmul M)
NI = P // IC      # 32 i-chunks
GRP = 4           # chunks per rowmin fold group
JW = 2048         # columns per psum tile (4 PSUM banks)
NJ = P // JW      # psum tiles per chunk
MMN = 512         # matmul moving free dim (1 bank fp32)
NB = P // IC      # 32 transpose blocks per colacc

SC = 16.0         # coordinate prescale (power of two)
BIG = 1.0e30
MIN_OP = mybir.AluOpType.min
ADD_OP = mybir.AluOpType.add
AX = mybir.AxisListType.X
SQRT = mybir.ActivationFunctionType.Sqrt


def _build_side(nc, wide, raw, coord_scale, mm_mode, placement, K):
    """Load one (P,2) point set and compute its operand row pieces into
    one wide [32, K*128] buffer (piece for operand row r at slice r).

    Wide layout: partition a holds points a*128..a*128+127. Returns the
    filled piece buffer. `placement` maps piece name -> list of rows.
    """
    ODT = F32 if mm_mode == "f32" else F16
    pb = wide.tile([32, K * IC], ODT, name=f"pb_{raw.name}")

    def sl(r):
        return pb[:, r * IC : (r + 1) * IC]

    def put(name, ap_writer):
        rows_ = placement[name]
        ap_writer(sl(rows_[0]))
        for r in rows_[1:]:
            nc.vector.tensor_copy(sl(r), sl(rows_[0]))

    # constant-one rows (adjacent by construction) via one memset
    ones_rows = placement["one"]
    nc.gpsimd.memset(
        pb[:, ones_rows[0] * IC : (ones_rows[-1] + 1) * IC], 1.0
    )

    xw = wide.tile([32, 2 * IC], F32, name=f"xw_{raw.name}")
    nc.sync.dma_start(xw[:], raw.rearrange("(a b) c -> a (b c)", a=32))
    xp = wide.tile([32, 2 * IC], F32, name=f"xp_{raw.name}")
    nc.scalar.mul(xp[:], xw[:], coord_scale)

    xpv = xp[:].rearrange("p (q c) -> p c q", c=2)   # [32, 2, 128]

    sq0 = wide.tile([32, IC], F32, name=f"sq0_{raw.name}")
    nc.vector.tensor_mul(sq0[:], xpv[:, 0, :], xpv[:, 0, :])
    sq1 = wide.tile([32, IC], F32, name=f"sq1_{raw.name}")
    nc.vector.tensor_mul(sq1[:], xpv[:, 1, :], xpv[:, 1, :])
    xn = wide.tile([32, IC], F32, name=f"xn_{raw.name}")
    nc.vector.tensor_add(xn[:], sq0[:], sq1[:])
    norm_fix = (SC / coord_scale) ** 2
    if norm_fix != 1.0:
        nc.scalar.mul(xn[:], xn[:], norm_fix)

    if mm_mode == "f32":
        put("c0", lambda d: nc.vector.tensor_copy(d, xpv[:, 0, :]))
        put("c1", lambda d: nc.vector.tensor_copy(d, xpv[:, 1, :]))
        put("n", lambda d: nc.vector.tensor_copy(d, xn[:]))
        return pb

    for c in (0, 1):
        put(f"c{c}h", lambda d, c=c: nc.scalar.copy(d, xpv[:, c, :]))
        h = sl(placement[f"c{c}h"][0])
        dd = wide.tile([32, IC], F32, name=f"c{c}d_{raw.name}")
        nc.vector.tensor_sub(dd[:], xpv[:, c, :], h)
        put(f"c{c}l", lambda d, dd=dd: nc.scalar.copy(d, dd[:]))
    put("nh", lambda d: nc.scalar.copy(d, xn[:]))
    nh = sl(placement["nh"][0])
    nd = wide.tile([32, IC], F32, name=f"nd_{raw.name}")
    nc.vector.tensor_sub(nd[:], xn[:], nh)
    put("nl", lambda d: nc.scalar.copy(d, nd[:]))
    return pb


class _DmaRR:
    """Round-robin DMAs over the SP HWDGE ring and the GPSIMD SWDGE
    ring. Never issues DMA from the scalar engine: a dma_start in the
    ACT instruction stream blocks the conversions behind it."""

    def __init__(self, nc):
        self.engines = [nc.sync, nc.scalar]
        self.i = 0

    def dma(self, out, in_, **kw):
        e = self.engines[self.i % len(self.engines)]
        self.i += 1
        e.dma_start(out, in_, **kw)


def _gather_side(nc, rr, dram_pool, pb, flat4, K):
    """Bounce the piece buffer through DRAM, landing it as flat operand
    rows [K, P] at partition strip 0, then replicate to strips
    32/64/96 for row-packed matmuls. Processed in two point-halves so
    the first half's chain completes early."""
    stage = dram_pool.tile([32, K * IC], pb.dtype, name=f"stage_{pb.name}")
    rr.dma(stage[:], pb[:])
    rr.dma(
        flat4[0:K, :],
        stage[:].rearrange("a (r q) -> r a q", r=K),
    )
    for t in range(1, 4):
        rr.dma(flat4[32 * t : 32 * t + K, :], flat4[0:K, :])


def build(mm_mode="fp16x2"):
    nc = bacc.Bacc(
        "TRN2", target_bir_lowering=False, debug=False, num_devices=NCORES
    )
    x = nc.dram_tensor("x", [P, 2], F32, kind="ExternalInput").ap()
    y = nc.dram_tensor("y", [P, 2], F32, kind="ExternalInput").ap()
    ident = nc.dram_tensor("ident", [IC, IC], BF16, kind="ExternalInput").ap()
    out_d = nc.dram_tensor("out", [1, 1], F32, kind="ExternalOutput").ap()

    K = 4 if mm_mode == "f32" else 10
    ODT = F32 if mm_mode == "f32" else F16
    NG = NI // GRP            # 8 groups
    HALF_G = NG // 2          # group index where colacc half 0 completes
    CM0_G = NG - 2            # emit half-0 reduce during this group

    with ExitStack() as ctx:
        tc = ctx.enter_context(tile.TileContext(nc))
        konst = ctx.enter_context(tc.tile_pool(name="konst", bufs=1))
        wide = ctx.enter_context(tc.tile_pool(name="wide", bufs=1))
        dram = ctx.enter_context(tc.tile_pool(name="dram", bufs=1, space="DRAM"))
        psum = ctx.enter_context(tc.tile_pool(name="psum", bufs=2, space="PSUM"))
        rows = ctx.enter_context(tc.tile_pool(name="rows", bufs=3))
        accp = ctx.enter_context(tc.tile_pool(name="accp", bufs=1))
        smalls = ctx.enter_context(tc.tile_pool(name="smalls", bufs=1))

        rr = _DmaRR(nc)

        # preload the sqrt activation table while input DMAs run
        dum = smalls.tile([1, 1], F32, name="dum")
        nc.gpsimd.memset(dum[:], 1.0)
        dum2 = smalls.tile([1, 1], F32, name="dum2")
        nc.scalar.activation(dum2[:], dum[:], SQRT)

        idt = konst.tile([IC, IC], BF16, name="idt")
        rr.dma(idt[:], ident)

        # operands replicated at partition strips 0/32/64/96 so four
        # small-K matmuls run concurrently in the PE's 32-row groups
        lhsT4 = konst.tile([IC, P], ODT, name="lhsT4")
        rhs4 = konst.tile([IC, P], ODT, name="rhs4")
        lhsT = lhsT4[0:K, :]
        rhs = rhs4[0:K, :]
        colacc = [accp.tile([IC, P], BF16, name=f"colacc{h}") for h in (0, 1)]
        nc.gpsimd.memset(colacc[0][:], BIG)
        nc.gpsimd.memset(colacc[1][:], BIG)
        rmins = smalls.tile([IC, NI], F32, name="rmins")

        if mm_mode == "f32":
            # lhsT rows: [x0, x1, xn, 1]; rhs rows: [s0, s1, 1, yn]
            lplace = {"c0": [0], "c1": [1], "n": [2], "one": [3]}
            rplace = {"c0": [0], "c1": [1], "one": [2], "n": [3]}
        else:
            # lhsT rows: [x0h x0h x0l  x1h x1h x1l  xnh xnl  1 1]
            # rhs  rows: [s0h s0l s0h  s1h s1l s1h  1   1    ynh ynl]
            lplace = {"c0h": [0, 1], "c0l": [2], "c1h": [3, 4], "c1l": [5],
                      "nh": [6], "nl": [7], "one": [8, 9]}
            rplace = {"c0h": [0, 2], "c0l": [1], "c1h": [3, 5], "c1l": [4],
                      "one": [6, 7], "nh": [8], "nl": [9]}
        xpb = _build_side(nc, wide, x, SC, mm_mode, lplace, K)
        ypb = _build_side(nc, wide, y, -2.0 * SC, mm_mode, rplace, K)
        _gather_side(nc, rr, dram, xpb, lhsT4, K)
        _gather_side(nc, rr, dram, ypb, rhs4, K)

        ct0 = accp.tile([IC, P], BF16, name="ct0")
        cm0 = smalls.tile([IC, NB], F32, name="cm0")

        # group schedule: 4-chunk groups, except the last two groups are
        # 2 chunks each so the end-of-loop DVE drain is shorter
        gsizes = [GRP] * (NI // GRP - 1) + [GRP // 2, GRP // 2]
        assert sum(gsizes) == NI

        # ---- main loop ----
        ic = 0
        for g, gs in enumerate(gsizes):
            g0 = ic
            drow = rows.tile([IC, GRP, P], BF16, name="drow")
            for cg in range(gs):
                half = (2 * ic) // NI
                for jc in range(NJ):
                    pt = psum.tile([IC, JW], F32, name="pt")
                    # 4 concurrent matmuls in distinct 32-row PE strips
                    for h in range(JW // MMN):
                        j0 = jc * JW + h * MMN
                        nc.tensor.matmul(
                            pt[:, h * MMN : (h + 1) * MMN],
                            lhsT4[32 * h : 32 * h + K,
                                  ic * IC : (ic + 1) * IC],
                            rhs4[32 * h : 32 * h + K, j0 : j0 + MMN],
                            start=True,
                            stop=True,
                            tile_position=(32 * h, 0),
                        )
                    nc.scalar.copy(
                        drow[:, cg, jc * JW : (jc + 1) * JW], pt[:]
                    )
                # column-min accumulation for this chunk (bf16 2x)
                nc.vector.tensor_tensor(
                    colacc[half][:], colacc[half][:], drow[:, cg, :], op=MIN_OP
                )
                ic += 1
                # half 0 complete -> xbar-transpose it on the DMA engines,
                # overlapping the second half of the loop
                if ic == NI // 2:
                    for b in range(NB):
                        bs = slice(b * IC, (b + 1) * IC)
                        nc.sync.dma_start(ct0[:, bs], colacc[0][:, bs],
                                          transpose=True)
            # row min: in-place halving fold tree over the whole group
            w = P
            while w > 32:
                w //= 2
                nc.vector.tensor_tensor(
                    drow[:, :gs, :w], drow[:, :gs, :w],
                    drow[:, :gs, w : 2 * w], op=MIN_OP,
                )
            nc.vector.tensor_reduce(
                rmins[:, g0 : g0 + gs], drow[:, :gs, :32],
                axis=AX, op=MIN_OP,
            )

        # ---- tail: half 0 reduce (transposes done long ago) runs on
        # DVE while the PE transposes half 1; half-1 reduces pipelined
        # in 8-block batches behind the transposes ----
        nc.vector.tensor_reduce(
            cm0[:], ct0[:].rearrange("p (b q) -> p b q", q=IC),
            axis=AX, op=MIN_OP,
        )
        ctb = psum.tile([IC, P], BF16, name="pt")
        cm1 = smalls.tile([IC, NB], F32, name="cm1")
        BB = 8
        for b0 in range(0, NB, BB):
            for b in range(b0, b0 + BB):
                bs = slice(b * IC, (b + 1) * IC)
                nc.tensor.transpose(ctb[:, bs], colacc[1][:, bs], idt[:])
            nc.vector.tensor_reduce(
                cm1[:, b0 : b0 + BB],
                ctb[:, b0 * IC : (b0 + BB) * IC].rearrange(
                    "p (b q) -> p b q", q=IC),
                axis=AX, op=MIN_OP,
            )
        cmin = smalls.tile([IC, NB], F32, name="cmin")
        nc.vector.tensor_tensor(cmin[:], cm0[:], cm1[:], op=MIN_OP)

        # ---- finalize: clamp, sqrt, row sums ----
        rclamp = smalls.tile([IC, NI], F32, name="rclamp")
        nc.vector.tensor_scalar_max(rclamp[:], rmins[:], 0.0)
        rsq = smalls.tile([IC, NI], F32, name="rsq")
        nc.scalar.activation(rsq[:], rclamp[:], SQRT)
        rsum = smalls.tile([IC, 1], F32, name="rsum")
        nc.vector.tensor_reduce(rsum[:], rsq[:], axis=AX, op=ADD_OP)

        cclamp = smalls.tile([IC, NB], F32, name="cclamp")
        nc.vector.tensor_scalar_max(cclamp[:], cmin[:], 0.0)
        csq = smalls.tile([IC, NB], F32, name="csq")
        nc.scalar.activation(csq[:], cclamp[:], SQRT)
        csum = smalls.tile([IC, 1], F32, name="csum")
        nc.vector.tensor_reduce(csum[:], csq[:], axis=AX, op=ADD_OP)

        # ---- combine: partition-sum via matmul with ones, then scale ----
        both = smalls.tile([IC, 1], F32, name="both")
        nc.vector.tensor_add(both[:], rsum[:], csum[:])
        ones = smalls.tile([IC, 1], F32, name="ones")
        nc.gpsimd.memset(ones[:], 1.0)
        fin_ps = psum.tile([IC, JW], F32, name="pt")
        nc.tensor.matmul(fin_ps[0:1, 0:1], both[:], ones[:], start=True,
                         stop=True)
        fin = smalls.tile([1, 1], F32, name="fin")
        nc.scalar.mul(fin[:], fin_ps[0:1, 0:1], 1.0 / (SC * 2.0 * P))
        nc.sync.dma_start(out_d, fin[:])

    nc.compile()
    return nc


_NC_CACHE = {}


def _get_nc(mm_mode):
    if mm_mode not in _NC_CACHE:
        _NC_CACHE[mm_mode] = build(mm_mode)
    return _NC_CACHE[mm_mode]


def _ident_np():
    import ml_dtypes
    return np.eye(IC, dtype=ml_dtypes.bfloat16)


def run(point_set_1, point_set_2, mm_mode=None, trace=False, tmpdir=None):
    """Run on 8 cores; returns ((8,) result, BassKernelResults)."""
    mm_mode = mm_mode or os.environ.get("CHAMFER_MM_MODE", "fp16x2")
    nc = _get_nc(mm_mode)
    x = np.ascontiguousarray(np.asarray(point_set_1), dtype=np.float32)
    y = np.ascontiguousarray(np.asarray(point_set_2), dtype=np.float32)
    assert x.shape == (NCORES, P, 2) and y.shape == (NCORES, P, 2)
    ident = _ident_np()
    in_maps = [{"x": x[c], "y": y[c], "ident": ident} for c in range(NCORES)]
    res = run_bass_kernel_spmd(
        nc, in_maps, list(range(NCORES)), trace=trace, tmpdir=tmpdir
    )
    out = np.array(
        [res.results[c]["out"][0, 0] for c in range(NCORES)], dtype=np.float32
    )
    return out, res


def kernel(point_set_1, point_set_2):
    out, _ = run(point_set_1, point_set_2)
    return out
